# revision 6
# baseline (speedup 1.0000x reference)
"""Trainium2 Bass kernel for AttentionBasedInflationBlock (spatial linear attention).

Data-parallel over the 8 (b*t) frames: one frame per NeuronCore.

Per-core math (frame x_f [n=9216, d=256], H=8 heads, dh=32):
    qkv = x @ w_qkv.T                (channel-major on device: qkvT[e, n])
    q   = softmax_dh(q) * dh^-0.5    (channel softmax -> ones-block matmul + recip + expand)
    k   = softmax_n(k)               (token softmax -> exp + ones-column in ctx matmul)
    ctx = k @ v.T per head           (32x32 per head, token-major ek/v via x-stationary matmul)
    out = ctx.T @ q per head         (block-diagonal ctx as lhsT)
    out = out @ w_out.T + b_out; out = out @ w_lin.T + b_lin; out += x

Everything on device is channel-major [channel, token]; host feeds x pre-transposed
and re-transposes the output. Matmuls run in bf16 (f32 PSUM accumulate); the
residual + final add stay f32.
"""

import os
from contextlib import ExitStack

import numpy as np
import ml_dtypes

import concourse.bass as bass
import concourse.tile as tile
import concourse.mybir as mybir
from concourse import bacc
from concourse.bass_utils import run_bass_kernel_spmd

HEADS = 8
D = 256
DH = D // HEADS           # 32
N = 9216                  # 96*96 tokens per frame
NT = 512                  # tokens per n-tile
NTILES = N // NT          # 18
SCALE = DH ** -0.5
F32 = mybir.dt.float32
BF16 = mybir.dt.bfloat16
BF16_NP = ml_dtypes.bfloat16


def _emit(tc, ctx, aps):
    nc = tc.nc
    EXP = mybir.ActivationFunctionType.Exp
    MULT = mybir.AluOpType.mult
    ADD = mybir.AluOpType.add

    singles = ctx.enter_context(tc.tile_pool(name="singles", bufs=1))
    work = ctx.enter_context(tc.tile_pool(name="work", bufs=3))
    psA = ctx.enter_context(tc.tile_pool(name="psA", bufs=3, space="PSUM"))
    pskv = ctx.enter_context(tc.tile_pool(name="pskv", bufs=2, space="PSUM"))
    pszq = ctx.enter_context(tc.tile_pool(name="pszq", bufs=1, space="PSUM"))
    psctx = ctx.enter_context(tc.tile_pool(name="psctx", bufs=1, space="PSUM"))

    # ---- resident weights ----
    wqkv_sb = singles.tile([128, 2, 768], BF16, tag="wqkv")
    nc.sync.dma_start(wqkv_sb[:], aps["wqkv"].rearrange("j p e -> p j e"))
    wout_sb = singles.tile([128, 2, 256], BF16, tag="wout")
    nc.sync.dma_start(wout_sb[:], aps["wout"].rearrange("j p e -> p j e"))
    wlin_sb = singles.tile([128, 2, 256], BF16, tag="wlin")
    nc.sync.dma_start(wlin_sb[:], aps["wlin"].rearrange("j p e -> p j e"))
    beff_sb = singles.tile([128, 2], F32, tag="beff")
    nc.sync.dma_start(beff_sb[:], aps["beff"])
    onesq_sb = singles.tile([128, 2, 128], BF16, tag="onesq")
    nc.sync.dma_start(onesq_sb[:], aps["onesq"].rearrange("j p r -> p j r"))
    onese_sb = singles.tile([128, 2, 128], BF16, tag="onese")
    nc.sync.dma_start(onese_sb[:], aps["onese"].rearrange("m r e -> r m e"))

    xt_ap = aps["xt"]
    xb_ap = aps["xb"]
    out_ap = aps["out"]

    # ---- pass 1: qkv, q-softmax (normalized eq), ctx accumulation ----
    ctxps = [psctx.tile([128, 257], F32, tag=f"ctx{m}", name=f"ctx{m}")
             for m in range(2)]
    eqn_chunks = []
    xf_chunks = []
    for it in range(NTILES):
        n0 = it * NT
        xf_t = singles.tile([128, 2, NT], F32, tag=f"xf{it}")
        nc.sync.dma_start(xf_t[:], xt_ap[:, :, n0:n0 + NT].rearrange("j p n -> p j n"))
        xf_chunks.append(xf_t)
        xb_t = work.tile([128, 2, NT], BF16, tag="xb")
        nc.sync.dma_start(xb_t[:], xb_ap[:, :, n0:n0 + NT].rearrange("j p n -> p j n"))

        # q (channel-major): psum[e_loc, tok] for the two 128-channel subtiles
        eqs = []
        for m in range(2):
            qp = psA.tile([128, NT], F32, tag="ps512")
            for j in range(2):
                nc.tensor.matmul(
                    qp[:], wqkv_sb[:, j, m * 128:(m + 1) * 128], xb_t[:, j, :],
                    start=(j == 0), stop=(j == 1))
            eq_t = work.tile([128, NT], BF16, tag="eq")
            nc.scalar.activation(eq_t[:], qp[:], EXP)
            eqs.append(eq_t)

        # zq replicated 16x: zqp[r, tok] = Zq[r // 16, tok] (full 128 partitions)
        zqp = pszq.tile([128, NT], F32, tag="zq")
        for j in range(2):
            nc.tensor.matmul(zqp[:], onesq_sb[:, j, :], eqs[j][:],
                             start=(j == 0), stop=(j == 1))
        rzq = work.tile([128, NT], BF16, tag="rzq")
        with nc.allow_low_precision("softmax normalizer in bf16; tol 2e-2"):
            nc.vector.reciprocal(rzq[:], zqp[:])

        # eqn = eq * broadcast(1/zq) (expand via (1/16)-ones matmul, K=128)
        eqn_t = singles.tile([128, 2, NT], BF16, tag=f"eqn{it}")
        for m in range(2):
            ep = psA.tile([128, NT], F32, tag="ps512")
            nc.tensor.matmul(ep[:], onese_sb[:, m, :], rzq[:],
                             start=True, stop=True)
            nc.vector.tensor_tensor(eqn_t[:, m, :], ep[:], eqs[m][:], MULT)
        eqn_chunks.append(eqn_t)

        # k,v token-major via x-stationary matmul; ctx accumulation
        for s in range(4):
            kvp = pskv.tile([128, NT], F32, tag="kv")
            for j in range(2):
                nc.tensor.matmul(
                    kvp[:], xb_t[:, j, s * 128:(s + 1) * 128], wqkv_sb[:, j, 256:768],
                    start=(j == 0), stop=(j == 1))
            ek_t = work.tile([128, 256], BF16, tag="ek")
            nc.scalar.activation(ek_t[:], kvp[:, 0:256], EXP)
            v1_t = work.tile([128, 257], BF16, tag="v1")
            nc.vector.tensor_copy(v1_t[:, 0:256], kvp[:, 256:512])
            nc.vector.memset(v1_t[:, 256:257], 1.0)
            first = (it == 0 and s == 0)
            last = (it == NTILES - 1 and s == 3)
            for m in range(2):
                nc.tensor.matmul(ctxps[m][:], ek_t[:, m * 128:(m + 1) * 128], v1_t[:],
                                 start=first, stop=last)

    # ---- inter-pass: normalize ctx rows by 1/Zk, fold q-softmax scale,
    #      assemble block-diagonal lhsT ----
    ctx_sb = []
    for m in range(2):
        rzk = work.tile([128, 1], F32, tag="rzk")
        nc.vector.reciprocal(rzk[:], ctxps[m][:, 256:257])
        nc.scalar.mul(rzk[:], rzk[:], SCALE)
        c_sb = singles.tile([128, 128], BF16, tag=f"ctxsb{m}")
        nc.vector.memset(c_sb[:], 0.0)
        for hh in range(4):
            ps = slice(32 * hh, 32 * hh + 32)
            fs = slice(128 * m + 32 * hh, 128 * m + 32 * hh + 32)
            nc.vector.tensor_scalar_mul(c_sb[ps, ps], ctxps[m][ps, fs], rzk[ps, :])
        ctx_sb.append(c_sb)

    # ---- pass 2: apply ctx, w_out, w_lin, bias + residual, store ----
    for it in range(NTILES):
        n0 = it * NT
        attn = []
        for m in range(2):
            up = psA.tile([128, NT], F32, tag="ps512")
            nc.tensor.matmul(up[:], ctx_sb[m][:], eqn_chunks[it][:, m, :],
                             start=True, stop=True)
            a_t = work.tile([128, NT], BF16, tag="attn")
            nc.any.tensor_copy(out=a_t[:], in_=up[:])
            attn.append(a_t)
        out2 = []
        for m in range(2):
            op2 = psA.tile([128, NT], F32, tag="ps512")
            for e in range(2):
                nc.tensor.matmul(op2[:], wout_sb[:, e, m * 128:(m + 1) * 128],
                                 attn[e][:], start=(e == 0), stop=(e == 1))
            o2_t = work.tile([128, NT], BF16, tag="o2")
            nc.any.tensor_copy(out=o2_t[:], in_=op2[:])
            out2.append(o2_t)
        for m in range(2):
            op3 = psA.tile([128, NT], F32, tag="ps512")
            for e in range(2):
                nc.tensor.matmul(op3[:], wlin_sb[:, e, m * 128:(m + 1) * 128],
                                 out2[e][:], start=(e == 0), stop=(e == 1))
            o_t = work.tile([128, NT], F32, tag="osb")
            nc.vector.scalar_tensor_tensor(o_t[:], op3[:], beff_sb[:, m:m + 1],
                                           xf_chunks[it][:, m, :], ADD, ADD)
            nc.sync.dma_start(out_ap[m, :, n0:n0 + NT], o_t[:])


def build_nc():
    nc = bacc.Bacc("TRN2", target_bir_lowering=False, debug=False,
                   enable_asserts=True)
    aps = {}
    aps["xt"] = nc.dram_tensor("xt", [2, 128, N], F32, kind="ExternalInput").ap()
    aps["xb"] = nc.dram_tensor("xb", [2, 128, N], BF16, kind="ExternalInput").ap()
    aps["wqkv"] = nc.dram_tensor("wqkv", [2, 128, 768], BF16, kind="ExternalInput").ap()
    aps["wout"] = nc.dram_tensor("wout", [2, 128, 256], BF16, kind="ExternalInput").ap()
    aps["wlin"] = nc.dram_tensor("wlin", [2, 128, 256], BF16, kind="ExternalInput").ap()
    aps["beff"] = nc.dram_tensor("beff", [128, 2], F32, kind="ExternalInput").ap()
    aps["onesq"] = nc.dram_tensor("onesq", [2, 128, 128], BF16,
                                  kind="ExternalInput").ap()
    aps["onese"] = nc.dram_tensor("onese", [2, 128, 128], BF16,
                                  kind="ExternalInput").ap()
    aps["out"] = nc.dram_tensor("out", [2, 128, N], F32, kind="ExternalOutput").ap()

    with tile.TileContext(nc) as tc:
        with ExitStack() as ctx:
            _emit(tc, ctx, aps)
    nc.compile()
    return nc


def make_in_maps(x, w_qkv, w_out, b_out, w_lin, b_lin):
    """Host-side sharding: one frame per core, channel-major x, replicated weights."""
    b, t, h, w, d = x.shape
    assert (b * t, h * w, d) == (8, N, D)
    xf = np.ascontiguousarray(x.reshape(8, N, D).transpose(0, 2, 1))  # [8, 256, 9216]
    xt = xf.reshape(8, 2, 128, N)
    xb = xt.astype(BF16_NP)

    wqkv_h = np.ascontiguousarray(w_qkv.T).reshape(2, 128, 768).astype(BF16_NP)
    wout_h = np.ascontiguousarray(w_out.T).reshape(2, 128, 256).astype(BF16_NP)
    wlin_h = np.ascontiguousarray(w_lin.T).reshape(2, 128, 256).astype(BF16_NP)
    beff = (w_lin.astype(np.float64) @ b_out.astype(np.float64)
            + b_lin.astype(np.float64)).astype(np.float32)
    beff_h = np.ascontiguousarray(beff.reshape(2, 128).T)  # [128, 2]

    # zq replication: zqp[r, n] = Zq[r//16, n];
    # onesq[j][d_loc, r] = 1 iff head(128j + d_loc) == r//16
    onesq = np.zeros((2, 128, 128), dtype=BF16_NP)
    for j in range(2):
        for p in range(128):
            h = 4 * j + p // 32
            onesq[j, p, 16 * h:16 * h + 16] = 1.0
    # expand: eqn row e of subtile m sums the 16 replicated recip rows * 1/16
    # onese[m][r, e_loc] = 1/16 iff r//16 == head(128m + e_loc)
    onese = np.zeros((2, 128, 128), dtype=BF16_NP)
    for m in range(2):
        for e in range(128):
            h = 4 * m + e // 32
            onese[m, 16 * h:16 * h + 16, e] = 1.0 / 16.0

    in_maps = []
    for f in range(8):
        in_maps.append({
            "xt": np.ascontiguousarray(xt[f]),
            "xb": np.ascontiguousarray(xb[f]),
            "wqkv": wqkv_h,
            "wout": wout_h,
            "wlin": wlin_h,
            "beff": beff_h,
            "onesq": onesq,
            "onese": onese,
        })
    return in_maps


def unshard(results, x):
    """results[f]["out"] is [2, 128, 9216] channel-major -> full [1,8,96,96,256]."""
    out = np.empty((8, N, D), dtype=np.float32)
    for f in range(8):
        out[f] = results[f]["out"].reshape(D, N).T
    return out.reshape(x.shape)


_NC_CACHE = []


def _get_nc():
    if not _NC_CACHE:
        _NC_CACHE.append(build_nc())
    return _NC_CACHE[0]


def run(inputs, trace=False, **spmd_kwargs):
    nc = _get_nc()
    in_maps = make_in_maps(**inputs)
    res = run_bass_kernel_spmd(nc, in_maps, core_ids=list(range(8)), trace=trace,
                               **spmd_kwargs)
    return unshard(res.results, inputs["x"]), res


def kernel(**inputs):
    out, _ = run(inputs, trace=False)
    return out
